# revision 6
# baseline (speedup 1.0000x reference)
"""Multi-head attention (B=4, S=2048, d_model=1024, H=16) on 8 TRN2 NeuronCores.

Sharding: core c handles batch c//2 and query rows [1024*(c%2), 1024*(c%2)+1024).
Each core redundantly projects K/V for its batch (no collectives needed) and
produces a disjoint [1024, 1024] slice of the output.

Per-core pipeline:
  phase V: V = v @ w_v + b_v in row layout [t, 16*65] (col 65h+64 := 1.0 so the
           attnV matmul's 65th output row accumulates sum(exp(scores)) for free)
  phase KQ: KT/QT in channel-major layout per head pair (fp32r matmuls)
  attention (per pair, per 512-query block): scoresT = K_h @ Q_h^T with the two
           heads of a pair run concurrently on disjoint PE row groups (K=64);
           exp on ScalarE (1/sqrt(d_k) folded into the activation scale);
           attnV with M=65 accumulating over 16 key chunks; unnormalized
           outputs + softmax denominators spill to DRAM
  epilogue: one dense 128-lane reciprocal of all 16K denominators (via DRAM
           reshape bounce); K=2 ones-matmul broadcasts recips across
           partitions; normalize; out-projection (fp32r) + bias; DMA out.
"""

import numpy as np

import bass_rust
import concourse.bass as bass
import concourse.mybir as mybir
import concourse.tile as tile
from concourse.bass_utils import run_bass_kernel_spmd
from concourse.vector_clock import ScopedClock

F32 = mybir.dt.float32
F32R = mybir.dt.float32r
AF = mybir.ActivationFunctionType
ADD = mybir.AluOpType.add
MULT = mybir.AluOpType.mult

D_MODEL = 1024
B = 4
S = 2048
N_CORES = 8
QL = 1024  # query rows per core
NPAIR = 8  # head pairs
NK = D_MODEL // 128  # contraction chunks
NT = S // 128  # key chunks
VPW = 65 * 16  # padded V width

# ---------------------------------------------------------------------------
# Workaround for this container's walrus build: each instruction may carry at
# most ONE embedded sync-wait ("Too many sync wait commands" otherwise). Tile
# attaches several; split the extras onto same-engine NOPs placed immediately
# before the instruction (engine queues are in-order => identical semantics).
_MAX_WAITS = 1


def _patched_lower(self, ordered):
    nc = self.nc
    for bb_name, insts in ordered.items():
        new_list = []
        for inst in insts:
            si = inst.sync_info
            waits = list(si.on_wait) if si is not None and si.on_wait else []
            if len(waits) > _MAX_WAITS:
                updates = list(si.on_update) if si.on_update else []
                for w in waits[:-_MAX_WAITS]:
                    nop = bass_rust.InstNoOp(
                        name=nc.get_next_instruction_name(),
                        engine=inst.engine,
                        debug=inst.debug,
                        sync_info=bass_rust.SyncInfo(on_wait=[w], on_update=[]),
                    )
                    new_list.append(nop)
                inst.sync_info = bass_rust.SyncInfo(
                    on_wait=waits[-_MAX_WAITS:], on_update=updates
                )
            new_list.append(inst)
        insts[:] = new_list
    return tile.TileContext._orig_lower_ordered_insts(self, ordered)


def _patched_drain(self, tick_clock, wait_clock):
    probe = self.nc.sync.nop(nofuse=True)
    wait_clock.add_sem_waits(probe.ins, ScopedClock({None: tick_clock.global_clock}))
    si = probe.ins.sync_info
    waits = list(si.on_wait) if si is not None and si.on_wait else []
    if len(waits) > _MAX_WAITS:
        probe.ins.sync_info = bass_rust.SyncInfo(
            on_wait=waits[:_MAX_WAITS], on_update=[]
        )
        for w in waits[_MAX_WAITS:]:
            n = self.nc.sync.nop(nofuse=True)
            n.ins.sync_info = bass_rust.SyncInfo(on_wait=[w], on_update=[])
    self.nc.sync.drain()
    self.nc.all_engine_barrier()
    assert self.sems is not None
    popped = self.nc._tile_sem_poison_stack.pop()
    assert popped is self._sem_poison
    self.nc.clear_and_free_semaphores(list(self.sems.allocated().values()))
    self.nc.all_engine_barrier()


def _install_patch():
    if not hasattr(tile.TileContext, "_orig_lower_ordered_insts"):
        tile.TileContext._orig_lower_ordered_insts = (
            tile.TileContext._lower_ordered_insts
        )
        tile.TileContext._lower_ordered_insts = _patched_lower
        tile.TileContext._drain_and_barrier = _patched_drain


# ---------------------------------------------------------------------------


def _build_bass():
    nc = bass.Bass()
    qt = nc.dram_tensor("qt", [D_MODEL, QL], F32R, kind="ExternalInput")
    kt = nc.dram_tensor("kt", [D_MODEL, S], F32R, kind="ExternalInput")
    vt = nc.dram_tensor("vt", [D_MODEL, S], F32R, kind="ExternalInput")
    wq = nc.dram_tensor("wq", [D_MODEL, D_MODEL], F32R, kind="ExternalInput")
    wk = nc.dram_tensor("wk", [D_MODEL, D_MODEL], F32R, kind="ExternalInput")
    wv = nc.dram_tensor("wv", [D_MODEL, D_MODEL], F32R, kind="ExternalInput")
    wo = nc.dram_tensor("wo", [D_MODEL, D_MODEL], F32R, kind="ExternalInput")
    bqt = nc.dram_tensor("bqt", [128, NK], F32, kind="ExternalInput")
    bkt = nc.dram_tensor("bkt", [128, NK], F32, kind="ExternalInput")
    bvr = nc.dram_tensor("bvr", [128, D_MODEL], F32, kind="ExternalInput")
    bor = nc.dram_tensor("bor", [128, D_MODEL], F32, kind="ExternalInput")
    ones2 = nc.dram_tensor("ones2", [128, 128], F32R, kind="ExternalInput")
    vones = nc.dram_tensor("vones", [128, 16], F32R, kind="ExternalInput")
    out = nc.dram_tensor("out", [QL, D_MODEL], F32, kind="ExternalOutput")
    xau = nc.dram_tensor("xau", [D_MODEL, QL], F32)  # unnormalized X_attn^T
    sums_d = nc.dram_tensor("sums_d", [128, 128], F32)
    sums_r = nc.dram_tensor("sums_r", [128, 128], F32)

    with tile.TileContext(nc) as tc:
        _emit(nc, tc, locals())
    return nc


def _emit(nc, tc, t):
    qt, kt, vt = t["qt"], t["kt"], t["vt"]
    wq, wk, wv, wo = t["wq"], t["wk"], t["wv"], t["wo"]
    bqt, bkt, bvr, bor = t["bqt"], t["bkt"], t["bvr"], t["bor"]
    ones2, out, vones = t["ones2"], t["out"], t["vones"]
    xau, sums_d, sums_r = t["xau"], t["sums_d"], t["sums_r"]

    P = tc.tile_pool

    with (
        P(name="consts", bufs=1) as consts,
        P(name="stg", bufs=2) as stg,
        P(name="psacc", bufs=1, space="PSUM") as psacc,
    ):
        ones_t = consts.tile([128, 128], F32R, tag="ones2")
        nc.sync.dma_start(ones_t[:], ones2[:])
        bqt_t = consts.tile([128, NK], F32, tag="bqt")
        nc.sync.dma_start(bqt_t[:], bqt[:])
        bkt_t = consts.tile([128, NK], F32, tag="bkt")
        nc.sync.dma_start(bkt_t[:], bkt[:])
        bvr_t = consts.tile([128, D_MODEL], F32, tag="bvr")
        nc.sync.dma_start(bvr_t[:], bvr[:])
        bor_t = consts.tile([128, D_MODEL], F32, tag="bor")
        nc.sync.dma_start(bor_t[:], bor[:])

        with P(name="psS", bufs=2, space="PSUM") as psS, P(name="pv", bufs=1) as pv:
            # ---- V projection -------------------------------------------
            v_tiles = []
            for c in range(NT):
                v = pv.tile([128, VPW], F32R, tag=f"v{c}")
                nc.sync.dma_start(
                    v[:, :].rearrange("p (h w) -> p h w", w=65)[:, :, 64:65],
                    vones[:, :, None],
                )
                v_tiles.append(v)

            with (
                P(name="wvp", bufs=1) as wvp,
                P(name="vstr", bufs=2) as vstr,
                P(name="psV", bufs=2, space="PSUM") as psV,
            ):
                wv_tiles = []
                for k in range(NK):
                    wvt = wvp.tile([128, D_MODEL], F32R, tag=f"wv{k}")
                    nc.sync.dma_start(wvt[:], wv[128 * k : 128 * k + 128, :])
                    wv_tiles.append(wvt)
                for c in range(NT):
                    vts = vstr.tile([128, 1024], F32R, tag="vts")
                    nc.sync.dma_start(
                        vts[:, :].rearrange("p (k f) -> p k f", k=NK),
                        vt[:, 128 * c : 128 * c + 128].rearrange(
                            "(k p) f -> p k f", k=NK
                        ),
                    )
                    for ob in range(2):
                        ps = psV.tile([128, 512], F32, tag="vproj")
                        for k in range(NK):
                            nc.tensor.matmul(
                                ps[:],
                                vts[:, 128 * k : 128 * k + 128],
                                wv_tiles[k][:, 512 * ob : 512 * ob + 512],
                                start=(k == 0),
                                stop=(k == NK - 1),
                            )
                        dst = v_tiles[c][:, :].rearrange("p (h w) -> p h w", w=65)[
                            :, 8 * ob : 8 * ob + 8, 0:64
                        ]
                        nc.vector.tensor_tensor(
                            dst,
                            ps[:, :].rearrange("p (h w) -> p h w", w=64),
                            bvr_t[:, 512 * ob : 512 * ob + 512].rearrange(
                                "p (h w) -> p h w", w=64
                            ),
                            ADD,
                        )

            # ---- K / Q projections --------------------------------------
            with P(name="pkq", bufs=1) as pkq:
                KT = [pkq.tile([128, S], F32R, name=f"ktg{g}", tag=f"ktg{g}") for g in range(NPAIR)]
                QT = [pkq.tile([128, QL], F32R, name=f"qtg{g}", tag=f"qtg{g}") for g in range(NPAIR)]

                with (
                    P(name="kstr", bufs=1) as kstr,
                    P(name="wks", bufs=2) as wks,
                    P(name="psP", bufs=2, space="PSUM") as psP,
                ):
                    for qtr in range(4):
                        t0 = 512 * qtr
                        kq = []
                        for k in range(NK):
                            ktile = kstr.tile([128, 512], F32R, tag=f"kth{k}")
                            nc.sync.dma_start(
                                ktile[:], kt[128 * k : 128 * k + 128, t0 : t0 + 512]
                            )
                            kq.append(ktile)
                        for g in range(NPAIR):
                            wkg = []
                            for k in range(NK):
                                wkt = wks.tile([128, 128], F32R, tag=f"wks{k}")
                                nc.sync.dma_start(
                                    wkt[:],
                                    wk[128 * k : 128 * k + 128, 128 * g : 128 * g + 128],
                                )
                                wkg.append(wkt)
                            ps = psP.tile([128, 512], F32, tag="kproj")
                            for k in range(NK):
                                nc.tensor.matmul(
                                    ps[:],
                                    wkg[k][:],
                                    kq[k][:],
                                    start=(k == 0),
                                    stop=(k == NK - 1),
                                )
                            nc.vector.tensor_scalar_add(
                                KT[g][:, t0 : t0 + 512], ps[:], bkt_t[:, g : g + 1]
                            )

                with (
                    P(name="qstr", bufs=1) as qstr,
                    P(name="wqs", bufs=2) as wqs,
                    P(name="psQ", bufs=2, space="PSUM") as psQ,
                ):
                    for qh in range(2):
                        q0 = 512 * qh
                        qq = []
                        for k in range(NK):
                            qtile = qstr.tile([128, 512], F32R, tag=f"qth{k}")
                            nc.sync.dma_start(
                                qtile[:], qt[128 * k : 128 * k + 128, q0 : q0 + 512]
                            )
                            qq.append(qtile)
                        for g in range(NPAIR):
                            wqg = []
                            for k in range(NK):
                                wqt = wqs.tile([128, 128], F32R, tag=f"wqs{k}")
                                nc.sync.dma_start(
                                    wqt[:],
                                    wq[128 * k : 128 * k + 128, 128 * g : 128 * g + 128],
                                )
                                wqg.append(wqt)
                            ps = psQ.tile([128, 512], F32, tag="qproj")
                            for k in range(NK):
                                nc.tensor.matmul(
                                    ps[:],
                                    wqg[k][:],
                                    qq[k][:],
                                    start=(k == 0),
                                    stop=(k == NK - 1),
                                )
                            nc.vector.tensor_scalar_add(
                                QT[g][:, q0 : q0 + 512], ps[:], bqt_t[:, g : g + 1]
                            )

                # ---- attention ------------------------------------------
                sums_flat = sums_d[:, :].rearrange("p f -> (p f)")
                with P(name="expp", bufs=6) as expp:
                    for g in range(NPAIR):
                        ktg, qtg = KT[g], QT[g]
                        for qb in range(2):
                            q0 = 512 * qb
                            acc = [
                                psacc.tile([65, 512], F32, name="acca", tag="acca"),
                                psacc.tile([65, 512], F32, name="accb", tag="accb"),
                            ]
                            for cg in range(NT // 2):
                                exps = []
                                for h in range(2):
                                    p0 = 64 * h
                                    sc = psS.tile([128, 1024], F32, tag="scores")
                                    for ci in range(2):
                                        c = 2 * cg + ci
                                        nc.tensor.matmul(
                                            sc[:, 512 * ci : 512 * ci + 512],
                                            ktg[p0 : p0 + 64, 128 * c : 128 * c + 128],
                                            qtg[p0 : p0 + 64, q0 : q0 + 512],
                                            start=True,
                                            stop=True,
                                            skip_group_check=True,
                                        )
                                    ex = expp.tile([128, 1024], F32R, tag="exp")
                                    nc.scalar.activation(ex[:], sc[:], AF.Exp, scale=0.125)
                                    exps.append(ex)
                                for h in range(2):
                                    hh = 2 * g + h
                                    for ci in range(2):
                                        c = 2 * cg + ci
                                        nc.tensor.matmul(
                                            acc[h][:],
                                            v_tiles[c][:, 65 * hh : 65 * hh + 65],
                                            exps[h][:, 512 * ci : 512 * ci + 512],
                                            start=(c == 0),
                                            stop=(c == NT - 1),
                                            skip_group_check=True,
                                        )
                            for h in range(2):
                                sg = stg.tile([65, 512], F32, tag="spill")
                                nc.vector.tensor_copy(sg[:], acc[h][0:65, :])
                                nc.sync.dma_start(
                                    xau[
                                        128 * g + 64 * h : 128 * g + 64 * h + 64,
                                        q0 : q0 + 512,
                                    ],
                                    sg[0:64, :],
                                )
                                base = g * 2048 + h * 1024 + qb * 512
                                nc.sync.dma_start(
                                    sums_flat[base : base + 512][None, :],
                                    sg[64:65, :],
                                )

        # ---- dense reciprocal of all sums --------------------------------
        den = stg.tile([128, 128], F32, tag="dense")
        nc.sync.dma_start(den[:], sums_d[:])
        denr = stg.tile([128, 128], F32, tag="denser")
        nc.vector.reciprocal(denr[:], den[:])
        nc.sync.dma_start(sums_r[:], denr[:])
        sums_r_flat = sums_r[:, :].rearrange("p f -> (p f)")

        # ---- normalize + output projection -------------------------------
        with (
            P(name="pxn", bufs=1) as pxn,
            P(name="pwo", bufs=1) as pwo,
            P(name="ph3s", bufs=2) as ph3s,
            P(name="ps3", bufs=1, space="PSUM") as ps3,
            P(name="ps3o", bufs=2, space="PSUM") as ps3o,
        ):
            wo_tiles = []
            for k in range(NK):
                wot = pwo.tile([128, D_MODEL], F32R, tag=f"wo{k}")
                nc.sync.dma_start(wot[:], wo[128 * k : 128 * k + 128, :])
                wo_tiles.append(wot)

            xn_tiles = []
            for g in range(NPAIR):
                srr = ph3s.tile([128, QL], F32R, tag="srr")
                for h in range(2):
                    base = g * 2048 + h * 1024
                    nc.gpsimd.dma_start(
                        srr[64 + h : 65 + h, :],
                        sums_r_flat[base : base + QL][None, :],
                    )
                xr = ph3s.tile([128, QL], F32, tag="xr")
                nc.sync.dma_start(xr[:], xau[128 * g : 128 * g + 128, :])
                rep = ps3.tile([128, QL], F32, tag="rep")
                for qb in range(2):
                    nc.tensor.matmul(
                        rep[:, 512 * qb : 512 * qb + 512],
                        ones_t[64:66, :],
                        srr[64:66, 512 * qb : 512 * qb + 512],
                        start=True,
                        stop=True,
                        skip_group_check=True,
                    )
                xn = pxn.tile([128, QL], F32R, tag=f"xn{g}")
                nc.vector.tensor_tensor(xn[:], xr[:], rep[:], MULT)
                xn_tiles.append(xn)

            for m in range(QL // 128):
                for ob in range(2):
                    ps = ps3o.tile([128, 512], F32, tag="oproj")
                    for g in range(NPAIR):
                        nc.tensor.matmul(
                            ps[:],
                            xn_tiles[g][:, 128 * m : 128 * m + 128],
                            wo_tiles[g][:, 512 * ob : 512 * ob + 512],
                            start=(g == 0),
                            stop=(g == NPAIR - 1),
                        )
                    ot = stg.tile([128, 512], F32, tag="outs")
                    nc.vector.tensor_tensor(
                        ot[:], ps[:], bor_t[:, 512 * ob : 512 * ob + 512], ADD
                    )
                    nc.sync.dma_start(
                        out[128 * m : 128 * m + 128, 512 * ob : 512 * ob + 512],
                        ot[:],
                    )


_NC_CACHE = None
LAST_RESULT = None


def _get_nc():
    global _NC_CACHE
    if _NC_CACHE is None:
        _install_patch()
        _NC_CACHE = _build_bass()
    return _NC_CACHE


def kernel(q, k, v, w_q, b_q, w_k, b_k, w_v, b_v, w_o, b_o):
    global LAST_RESULT
    q = np.asarray(q, np.float32)
    k = np.asarray(k, np.float32)
    v = np.asarray(v, np.float32)
    w_q = np.ascontiguousarray(np.asarray(w_q, np.float32))
    w_k = np.ascontiguousarray(np.asarray(w_k, np.float32))
    w_v = np.ascontiguousarray(np.asarray(w_v, np.float32))
    w_o = np.ascontiguousarray(np.asarray(w_o, np.float32))
    b_q = np.asarray(b_q, np.float32)
    b_k = np.asarray(b_k, np.float32)
    b_v = np.asarray(b_v, np.float32)
    b_o = np.asarray(b_o, np.float32)

    bqt = np.ascontiguousarray(b_q.reshape(NK, 128).T)
    bkt = np.ascontiguousarray(b_k.reshape(NK, 128).T)
    bvr = np.ascontiguousarray(np.broadcast_to(b_v[None, :], (128, D_MODEL)))
    bor = np.ascontiguousarray(np.broadcast_to(b_o[None, :], (128, D_MODEL)))
    ones2 = np.zeros((128, 128), np.float32)
    ones2[64, 0:64] = 1.0
    ones2[65, 64:128] = 1.0
    vones_np = np.ones((128, 16), np.float32)

    in_maps = []
    for c in range(N_CORES):
        b = c // 2
        r0 = QL * (c % 2)
        in_maps.append(
            {
                "qt": np.ascontiguousarray(q[b, r0 : r0 + QL, :].T),
                "kt": np.ascontiguousarray(k[b].T),
                "vt": np.ascontiguousarray(v[b].T),
                "wq": w_q,
                "wk": w_k,
                "wv": w_v,
                "wo": w_o,
                "bqt": bqt,
                "bkt": bkt,
                "bvr": bvr,
                "bor": bor,
                "ones2": ones2,
                "vones": vones_np,
            }
        )

    nc = _get_nc()
    res = run_bass_kernel_spmd(nc, in_maps, list(range(N_CORES)))
    LAST_RESULT = res

    outp = np.empty((B, S, D_MODEL), np.float32)
    for c in range(N_CORES):
        b = c // 2
        r0 = QL * (c % 2)
        outp[b, r0 : r0 + QL, :] = res.results[c]["out"]
    return outp
